# revision 31
# baseline (speedup 1.0000x reference)
# Trainium2 Bass kernel for vq_codebook problem (nn_DULLI2_21869973471276).
# Strategy: data-parallel over batch (2 images per core, 8 cores).
# All matmuls use split2-bf16 (hi/lo bf16 decomposition, 3 accumulating
# PE passes) which measures at fp32-equivalent accuracy on HW at 3x bf16 cost.
import os
import functools
import numpy as np
import ml_dtypes

BF = ml_dtypes.bfloat16
B, C1, H, W = 16, 384, 40, 40
K0, C2, NCODE = 2048, 2048, 27
NPX = H * W  # 1600
N_CORES = int(os.environ.get("K_NCORES", "8"))
IMGS = 2  # images per core

# pixel chunking for VQ0 (128-partition chunks)
PCS = [(i * 128, min(128, NPX - i * 128)) for i in range((NPX + 127) // 128)]  # 13
N_BANDS = 2           # conv row bands per image (20 rows each)
BAND_PX = NPX // N_BANDS   # 800
BAND_ROWS = H // N_BANDS   # 20
CONV_NCH = 2          # 400-px chunks per band
CNPX = BAND_PX // CONV_NCH  # 400
CROWS = CNPX // W     # 10 rows per conv chunk
SUBS = [(0, 128), (128, 128), (256, 128), (384, 16)]  # px subchunks within 400


def _split2(a):
    a = np.asarray(a, np.float32)
    hi = a.astype(BF)
    lo = (a - hi.astype(np.float32)).astype(BF)
    return hi, lo


def _split3(z):
    # z: float64 -> 3 bf16 terms summing to z with ~2^-24 rel error
    t1 = z.astype(BF)
    r = z - t1.astype(np.float64)
    t2 = r.astype(BF)
    t3 = (r - t2.astype(np.float64)).astype(BF)
    return np.stack([t1, t2, t3]).astype(BF)


@functools.lru_cache(maxsize=1)
def _program():
    import concourse.tile as tile
    from concourse import bacc, mybir, masks

    f32 = mybir.dt.float32
    bfl = mybir.dt.bfloat16
    EXP = mybir.ActivationFunctionType.Exp
    AXX = mybir.AxisListType.X
    from contextlib import ExitStack

    nc = bacc.Bacc("TRN2", target_bir_lowering=False, debug=False,
                   enable_asserts=True, num_devices=1)

    def din(name, shape, dt=bfl):
        return nc.dram_tensor(name, list(shape), dt, kind="ExternalInput").ap()

    def dout(name, shape, dt=f32):
        return nc.dram_tensor(name, list(shape), dt, kind="ExternalOutput").ap()

    xh_d = din("xh", [IMGS, 3, 128, NPX])
    xl_d = din("xl", [IMGS, 3, 128, NPX])
    vq0t_h_d = din("vq0t_h", [128, 3, K0])
    vq0t_l_d = din("vq0t_l", [128, 3, K0])
    vq0n_h_d = din("vq0n_h", [128, 16, C1])
    vq0n_l_d = din("vq0n_l", [128, 16, C1])
    z0_d = din("z0", [1, 3, K0])
    w1_h_d = din("w1_h", [16, 128, 36, 128])
    w1_l_d = din("w1_l", [16, 128, 36, 128])
    b1_d = din("b1c", [128, 16], mybir.dt.float32)
    w2_h_d = din("w2_h", [16, 128, 16, 128])
    w2_l_d = din("w2_l", [16, 128, 16, 128])
    b2_d = din("b2s", [1, 2, C2])
    vq1t_h_d = din("vq1t_h", [128, 16, NCODE])
    vq1t_l_d = din("vq1t_l", [128, 16, NCODE])
    vq1n_h_d = din("vq1n_h", [NCODE, C2])
    vq1n_l_d = din("vq1n_l", [NCODE, C2])
    z1_d = din("z1", [1, 3, NCODE])

    qx0_o = dout("qx0o", [IMGS, 3, 128, NPX])
    a0_o = dout("a0o", [IMGS, 128, 16, NPX])
    x3_o = dout("x3o", [IMGS, 16, 128, NPX])
    qx3_o = dout("qx3o", [IMGS, 128, 16, NPX])
    a3_o = dout("a3o", [IMGS, NCODE, NPX])

    with tile.TileContext(nc) as tc, ExitStack() as octx:
        g = octx.enter_context(tc.tile_pool(name="glob", bufs=1))
        ident = g.tile([128, 128], f32)
        masks.make_identity(nc, ident[:])
        ones_bf = g.tile([1, 400], bfl)
        nc.vector.memset(ones_bf[:], 1.0)
        z0sb = g.tile([1, 3, K0], bfl)
        nc.sync.dma_start(z0sb[:], z0_d[:])
        z1sb = g.tile([1, 3, NCODE], bfl)
        nc.sync.dma_start(z1sb[:], z1_d[:])
        vq1t_h = g.tile([128, 16, NCODE], bfl)
        nc.sync.dma_start(vq1t_h[:], vq1t_h_d[:])
        vq1t_l = g.tile([128, 16, NCODE], bfl)
        nc.sync.dma_start(vq1t_l[:], vq1t_l_d[:])
        vq1n_h = g.tile([NCODE, C2], bfl)
        nc.sync.dma_start(vq1n_h[:], vq1n_h_d[:])
        vq1n_l = g.tile([NCODE, C2], bfl)
        nc.sync.dma_start(vq1n_l[:], vq1n_l_d[:])

        for img in range(IMGS):
            with tc.tile_pool(name="pimg", bufs=1) as pimg:
                qxpf = [pimg.tile([128, 42, 42], f32, name=f"qxpf{k}") for k in range(3)]
                a3sb = pimg.tile([NCODE, NPX], f32)
                b1c = pimg.tile([128, 16], f32)
                nc.sync.dma_start(b1c[:], b1_d[:])

                # ---------------- VQ0 ----------------
                with tc.tile_pool(name="vqA", bufs=1) as pA, \
                     tc.tile_pool(name="vqB", bufs=2) as pB, \
                     tc.tile_pool(name="psL", bufs=1, space="PSUM") as ppL, \
                     tc.tile_pool(name="psT", bufs=2, space="PSUM") as ppT, \
                     tc.tile_pool(name="psQ", bufs=2, space="PSUM") as ppQ:
                    vq0t_h = pA.tile([128, 3, K0], bfl)
                    nc.sync.dma_start(vq0t_h[:], vq0t_h_d[:])
                    vq0t_l = pA.tile([128, 3, K0], bfl)
                    nc.sync.dma_start(vq0t_l[:], vq0t_l_d[:])
                    vq0n_h = pA.tile([128, 16, C1], bfl)
                    nc.sync.dma_start(vq0n_h[:], vq0n_h_d[:])
                    vq0n_l = pA.tile([128, 16, C1], bfl)
                    nc.sync.dma_start(vq0n_l[:], vq0n_l_d[:])
                    xh = [pA.tile([128, NPX], bfl, name=f"xh{k}") for k in range(3)]
                    xl = [pA.tile([128, NPX], bfl, name=f"xl{k}") for k in range(3)]
                    for k in range(3):
                        nc.sync.dma_start(xh[k][:], xh_d[img, k])
                        nc.sync.dma_start(xl[k][:], xl_d[img, k])
                    qx0sb = [pA.tile([128, NPX], f32, name=f"qx0sb{k}") for k in range(3)]

                    for (px0, pw) in PCS:
                        Lps = [ppL.tile([128, 512], f32, name=f"L{gg}") for gg in range(4)]
                        for gg in range(4):
                            gs = slice(gg * 512, gg * 512 + 512)
                            for t in range(3):
                                nc.tensor.matmul(Lps[gg][:pw], ones_bf[:1, :pw],
                                                 z0sb[:1, t, gs],
                                                 start=(t == 0), stop=False)
                            i = 0
                            for (Lx, Rv) in ((xh, vq0t_h), (xh, vq0t_l), (xl, vq0t_h)):
                                for kch in range(3):
                                    i += 1
                                    nc.tensor.matmul(
                                        Lps[gg][:pw],
                                        Lx[kch][:, px0:px0 + pw],
                                        Rv[:, kch, gs],
                                        start=False, stop=(i == 9))
                        m4 = pB.tile([128, 4], f32, name="m4")
                        for gg in range(4):
                            nc.vector.reduce_max(m4[:pw, gg:gg + 1], Lps[gg][:pw], axis=AXX)
                        m1 = pB.tile([128, 1], f32, name="m1")
                        nc.vector.reduce_max(m1[:pw], m4[:pw], axis=AXX)
                        bias = pB.tile([128, 1], f32, name="bias")
                        nc.vector.tensor_scalar_mul(bias[:pw], m1[:pw], -200.0)
                        E32 = pB.tile([128, K0], f32, name="E32", bufs=1)
                        for gg in range(4):
                            nc.scalar.activation(E32[:pw, gg * 512:(gg + 1) * 512],
                                                 Lps[gg][:pw], EXP,
                                                 bias=bias[:pw], scale=200.0)
                        s = pB.tile([128, 1], f32, name="s")
                        nc.vector.reduce_sum(s[:pw], E32[:pw], axis=AXX)
                        r = pB.tile([128, 1], f32, name="r")
                        nc.vector.reciprocal(r[:pw], s[:pw])
                        P32 = E32
                        nc.vector.tensor_scalar_mul(P32[:pw], E32[:pw], r[:pw])
                        pci = px0 // 128
                        if pci % 4 == 0:
                            PhT = pB.tile([128, 16, 512], bfl, name="PhT", bufs=1)
                            PlT = pB.tile([128, 16, 512], bfl, name="PlT", bufs=1)
                            gpx0 = px0
                        go = px0 - gpx0  # column offset within group tile
                        a0sb = pB.tile([128, 16, 128], f32, name="a0sb", bufs=1)
                        for kk in range(16):
                            tp = ppT.tile([128, 128], f32, name="tp")
                            nc.tensor.transpose(tp[:, :pw],
                                                P32[:pw, kk * 128:(kk + 1) * 128],
                                                ident[:pw, :pw])
                            nc.scalar.copy(a0sb[:, kk, :pw], tp[:, :pw])
                            nc.vector.tensor_copy(PhT[:, kk, go:go + pw],
                                                  a0sb[:, kk, :pw])
                            nc.vector.tensor_sub(PlT[:, kk, go:go + pw],
                                                 a0sb[:, kk, :pw],
                                                 PhT[:, kk, go:go + pw])
                        nc.sync.dma_start(a0_o[img, :, :, px0:px0 + pw],
                                          a0sb[:, :, :pw])
                        if pci % 4 == 3 or px0 + pw == NPX:
                            gw = px0 + pw - gpx0
                            for c3 in range(3):
                                qps = ppQ.tile([128, 512], f32, name="qps")
                                i = 0
                                for (Lw, Rt) in ((vq0n_h, PhT), (vq0n_h, PlT),
                                                 (vq0n_l, PhT)):
                                    for kk in range(16):
                                        i += 1
                                        nc.tensor.matmul(
                                            qps[:, :gw],
                                            Lw[:, kk, c3 * 128:(c3 + 1) * 128],
                                            Rt[:, kk, :gw],
                                            start=(i == 1), stop=(i == 48))
                                nc.scalar.copy(qx0sb[c3][:, gpx0:gpx0 + gw],
                                               qps[:, :gw])

                    for c3 in range(3):
                        nc.sync.dma_start(qx0_o[img, c3], qx0sb[c3][:])
                        src3d = qx0sb[c3].rearrange("p (h w) -> p h w", h=H)
                        q = qxpf[c3]
                        nc.vector.tensor_copy(q[:, 1:41, 1:41], src3d)
                        nc.vector.tensor_copy(q[:, 0:1, 1:41], q[:, 2:3, 1:41])
                        nc.vector.tensor_copy(q[:, 41:42, 1:41], q[:, 39:40, 1:41])
                        nc.vector.tensor_copy(q[:, :, 0:1], q[:, :, 2:3])
                        nc.vector.tensor_copy(q[:, :, 41:42], q[:, :, 39:40])

                # ---------------- conv1 + conv2 + VQ1, per row band ----------------
                for band in range(N_BANDS):
                    bpx = band * BAND_PX
                    with tc.tile_pool(name="cbA", bufs=1) as cA, \
                         tc.tile_pool(name="cbW", bufs=2) as cW, \
                         tc.tile_pool(name="cbB", bufs=2) as cB, \
                         tc.tile_pool(name="psH", bufs=2, space="PSUM") as ppH, \
                         tc.tile_pool(name="psV", bufs=1, space="PSUM") as ppV:
                        hh = [cA.tile([128, BAND_PX], bfl, name=f"hh{m}") for m in range(16)]
                        hl = [cA.tile([128, BAND_PX], bfl, name=f"hl{m}") for m in range(16)]
                        # conv1 via 1D row-Winograd F(2,3)
                        with tc.tile_pool(name="wno", bufs=1) as pW:
                            Ehs = pW.tile([128, 3, 4, 10, 42], bfl)
                            Els = pW.tile([128, 3, 4, 10, 42], bfl)
                            r0 = band * BAND_ROWS
                            with tc.tile_pool(name="wE", bufs=1) as pE:
                              for kch in range(3):
                                Ew = pE.tile([128, 4, 10, 42], f32, name="Ew")
                                Xr = lambda r: qxpf[kch][:, r0 + r:min(r0 + r + 20, 42):2, :]
                                nc.vector.tensor_sub(Ew[:, 0], Xr(0), Xr(2))
                                nc.vector.tensor_add(Ew[:, 1], Xr(1), Xr(2))
                                nc.vector.tensor_sub(Ew[:, 2], Xr(2), Xr(1))
                                nc.vector.tensor_sub(Ew[:, 3], Xr(1), Xr(3))
                                nc.vector.tensor_copy(Ehs[:, kch], Ew[:])
                                nc.vector.tensor_sub(Els[:, kch], Ew[:], Ehs[:, kch])
                              pass
                            for mch in range(16):
                                wwh = cW.tile([128, 36, 128], bfl, name="wwh")
                                nc.sync.dma_start(wwh[:], w1_h_d[mch])
                                wwl = cW.tile([128, 36, 128], bfl, name="wwl")
                                nc.sync.dma_start(wwl[:], w1_l_d[mch])
                                Mu = [ppH.tile([128, 10, 40], f32, name=f"Mu{u}",
                                               bufs=1) for u in range(4)]
                                for u in range(4):
                                    i = 0
                                    for (Wt, Et) in ((wwh, Ehs), (wwh, Els),
                                                     (wwl, Ehs)):
                                        for dx in range(3):
                                            for kch in range(3):
                                                j = (u * 3 + dx) * 3 + kch
                                                i += 1
                                                nc.tensor.matmul(
                                                    Mu[u][:], Wt[:, j, :],
                                                    Et[:, kch, u, :, dx:dx + 40],
                                                    start=(i == 1), stop=(i == 27))
                                hv_h = hh[mch].rearrange("p (a b c) -> p a b c",
                                                         a=10, b=2, c=40)
                                hv_l = hl[mch].rearrange("p (a b c) -> p a b c",
                                                         a=10, b=2, c=40)
                                Yb = pW.tile([128, 10, 40], f32, name="Yb")
                                t01 = pW.tile([128, 10, 40], f32, name="t01w")
                                for dy in range(2):
                                    if dy == 0:
                                        nc.scalar.copy(Yb[:], Mu[0][:])
                                        nc.vector.tensor_add(Yb[:], Yb[:], Mu[1][:])
                                        nc.vector.tensor_add(Yb[:], Yb[:], Mu[2][:])
                                    else:
                                        nc.scalar.copy(Yb[:], Mu[1][:])
                                        nc.vector.tensor_sub(Yb[:], Yb[:], Mu[2][:])
                                        nc.vector.tensor_sub(Yb[:], Yb[:], Mu[3][:])
                                    nc.vector.tensor_scalar_add(Yb[:], Yb[:],
                                                                b1c[:, mch:mch + 1])
                                    nc.scalar.mul(t01[:], Yb[:], 0.1)
                                    nc.vector.tensor_max(t01[:], Yb[:], t01[:])
                                    nc.vector.tensor_copy(hv_h[:, :, dy, :], t01[:])
                                    nc.vector.tensor_sub(hv_l[:, :, dy, :], t01[:],
                                                         hv_h[:, :, dy, :])
                        # conv2 + VQ1 dist accumulation
                        b2sb = cB.tile([1, 2, C2], bfl, name="b2sb", bufs=1)
                        nc.sync.dma_start(b2sb[:], b2_d[:])
                        dacc = cA.tile([128, 8 * NCODE], f32)
                        for mch2 in range(16):
                            w2b_h = cW.tile([128, 16, 128], bfl, name="w2bh")
                            nc.sync.dma_start(w2b_h[:], w2_h_d[mch2])
                            w2b_l = cW.tile([128, 16, 128], bfl, name="w2bl")
                            nc.sync.dma_start(w2b_l[:], w2_l_d[mch2])
                            ms = slice(mch2 * 128, mch2 * 128 + 128)
                            for nch in range(CONV_NCH):
                                ns = slice(nch * CNPX, nch * CNPX + CNPX)
                                xps = ppH.tile([128, CNPX], f32, name="hps", bufs=1)
                                for t in range(2):
                                    nc.tensor.matmul(xps[:], b2sb[:1, t, ms],
                                                     ones_bf[:1, :CNPX],
                                                     start=(t == 0), stop=False)
                                i = 0
                                for (Wb, Hx) in ((w2b_h, hh), (w2b_h, hl), (w2b_l, hh)):
                                    for kch in range(16):
                                        i += 1
                                        nc.tensor.matmul(xps[:], Wb[:, kch, :],
                                                         Hx[kch][:, ns],
                                                         start=False, stop=(i == 48))
                                x3sb = cB.tile([128, CNPX], f32, name="x3sb")
                                nc.scalar.copy(x3sb[:], xps[:])
                                nc.sync.dma_start(
                                    x3_o[img, mch2, :, bpx + nch * CNPX:
                                         bpx + nch * CNPX + CNPX], x3sb[:])
                                x3h = cB.tile([128, CNPX], bfl, name="x3h")
                                nc.vector.tensor_copy(x3h[:], x3sb[:])
                                x3l = cB.tile([128, CNPX], bfl, name="x3l")
                                nc.vector.tensor_sub(x3l[:], x3sb[:], x3h[:])
                                dps = ppH.tile([128, 4 * NCODE], f32, name="dps", bufs=1)
                                for si, (s0, sw) in enumerate(SUBS):
                                    ds = slice(si * NCODE, si * NCODE + NCODE)
                                    if mch2 == 0:
                                        for t in range(3):
                                            nc.tensor.matmul(
                                                dps[:sw, ds], ones_bf[:1, :sw],
                                                z1sb[:1, t, :],
                                                start=(t == 0), stop=False)
                                    i = 0
                                    for (Lx, Rv) in ((x3h, vq1t_h), (x3h, vq1t_l),
                                                     (x3l, vq1t_h)):
                                        i += 1
                                        nc.tensor.matmul(
                                            dps[:sw, ds],
                                            Lx[:, s0:s0 + sw],
                                            Rv[:, mch2, :],
                                            start=(i == 1 and mch2 != 0),
                                            stop=(i == 3))
                                    das = slice((nch * 4 + si) * NCODE,
                                                (nch * 4 + si) * NCODE + NCODE)
                                    if mch2 == 0:
                                        nc.scalar.copy(dacc[:sw, das], dps[:sw, ds])
                                    else:
                                        nc.vector.tensor_add(dacc[:sw, das],
                                                             dacc[:sw, das],
                                                             dps[:sw, ds])
                        # VQ1 softmax + q3 + outputs
                        for pc2 in range(8):
                            s0g = [0, 128, 256, 384, 400, 528, 656, 784][pc2]
                            pw = [128, 128, 128, 16, 128, 128, 128, 16][pc2]
                            ds = slice(pc2 * NCODE, pc2 * NCODE + NCODE)
                            m1 = cB.tile([128, 1], f32, name="m1v")
                            nc.vector.reduce_max(m1[:pw], dacc[:pw, ds], axis=AXX)
                            bias = cB.tile([128, 1], f32, name="biasv")
                            nc.vector.tensor_scalar_mul(bias[:pw], m1[:pw], -200.0)
                            E1 = cB.tile([128, NCODE], f32, name="E1")
                            nc.scalar.activation(E1[:pw], dacc[:pw, ds], EXP,
                                                 bias=bias[:pw], scale=200.0)
                            s1 = cB.tile([128, 1], f32, name="s1")
                            nc.vector.reduce_sum(s1[:pw], E1[:pw], axis=AXX)
                            r1 = cB.tile([128, 1], f32, name="r1")
                            nc.vector.reciprocal(r1[:pw], s1[:pw])
                            P1 = cB.tile([128, NCODE], f32, name="P1")
                            nc.vector.tensor_scalar_mul(P1[:pw], E1[:pw], r1[:pw])
                            tp1 = ppV.tile([NCODE, 128], f32, name="tp1")
                            nc.tensor.transpose(tp1[:, :pw], P1[:pw], ident[:pw, :pw])
                            pxa = bpx + s0g
                            nc.scalar.copy(a3sb[:, pxa:pxa + pw], tp1[:, :pw])
                            P1hT = cB.tile([NCODE, 128], bfl, name="P1hT")
                            nc.vector.tensor_copy(P1hT[:, :pw], tp1[:, :pw])
                            P1lT = cB.tile([NCODE, 128], bfl, name="P1lT")
                            nc.vector.tensor_sub(P1lT[:, :pw], a3sb[:, pxa:pxa + pw],
                                                 P1hT[:, :pw])
                            q3st = cB.tile([128, 16, 128], f32, name="q3st", bufs=1)
                            for c16 in range(16):
                                cs = slice(c16 * 128, c16 * 128 + 128)
                                q3ps = ppV.tile([128, 128], f32, name="q3ps")
                                i = 0
                                for (Lw, Rt) in ((vq1n_h, P1hT), (vq1n_h, P1lT),
                                                 (vq1n_l, P1hT)):
                                    i += 1
                                    nc.tensor.matmul(q3ps[:, :pw], Lw[:, cs],
                                                     Rt[:, :pw],
                                                     start=(i == 1), stop=(i == 3))
                                nc.scalar.copy(q3st[:, c16, :pw], q3ps[:, :pw])
                            nc.sync.dma_start(qx3_o[img, :, :, pxa:pxa + pw],
                                              q3st[:, :, :pw])
                nc.sync.dma_start(a3_o[img], a3sb[:])
    nc.compile()
    return nc


def _host_inputs(x0, vq0, vq1, w1, b1, w2, b2):
    x0 = np.asarray(x0, np.float32)
    vq0 = np.asarray(vq0, np.float32)
    vq1 = np.asarray(vq1, np.float32)
    w1 = np.asarray(w1, np.float32)
    b1 = np.asarray(b1, np.float32)
    w2 = np.asarray(w2, np.float32)
    b2 = np.asarray(b2, np.float32)

    vq0t_h, vq0t_l = _split2(vq0.T)                      # [384, 2048]
    vq0t_h = vq0t_h.reshape(3, 128, K0).transpose(1, 0, 2).copy()
    vq0t_l = vq0t_l.reshape(3, 128, K0).transpose(1, 0, 2).copy()
    vq0n_h, vq0n_l = _split2(vq0)                        # [2048, 384]
    vq0n_h = vq0n_h.reshape(16, 128, C1).transpose(1, 0, 2).copy()
    vq0n_l = vq0n_l.reshape(16, 128, C1).transpose(1, 0, 2).copy()
    z0 = _split3(-0.5 * (vq0.astype(np.float64) ** 2).sum(1))   # [3, 2048]

    G = np.array([[1, 0, 0], [.5, .5, .5], [.5, -.5, .5], [0, 0, 1]], np.float64)
    U = np.einsum('ud,oidx->oiux', G, w1.astype(np.float64))  # [O, I, 4u, 3dx]
    Uh = U.astype(BF)
    Ul = (U - Uh.astype(np.float64)).astype(BF)
    # -> [mch, k, (u*3+dx)*3+kch, m]
    def w1fmt(a):
        a = np.asarray(a, BF).transpose(1, 0, 2, 3)      # [I, O, 4, 3]
        a = a.reshape(3, 128, 16, 128, 12)               # [kch,k,mch,m,udx]
        return a.transpose(2, 1, 4, 0, 3).reshape(16, 128, 36, 128).copy()
    w1h, w1l = w1fmt(Uh), w1fmt(Ul)
    b1c = np.ascontiguousarray(b1.reshape(16, 128).T)    # [128, 16] f32

    w2tr = w2[:, :, 0, 0].T                              # [in 2048, out 2048]
    w2h, w2l = _split2(w2tr)
    def w2fmt(a):
        a = np.asarray(a, BF).reshape(16, 128, 16, 128).transpose(2, 1, 0, 3)
        return a.copy()
    w2h, w2l = w2fmt(w2h), w2fmt(w2l)
    b2s = np.stack(_split2(b2))

    vq1t_h, vq1t_l = _split2(vq1.T)                      # [2048, 27]
    vq1t_h = vq1t_h.reshape(16, 128, NCODE).transpose(1, 0, 2).copy()
    vq1t_l = vq1t_l.reshape(16, 128, NCODE).transpose(1, 0, 2).copy()
    vq1n_h, vq1n_l = _split2(vq1)                        # [27, 2048]
    z1 = _split3(-0.5 * (vq1.astype(np.float64) ** 2).sum(1))   # [3, 27]

    shared = dict(vq0t_h=vq0t_h, vq0t_l=vq0t_l, vq0n_h=vq0n_h, vq0n_l=vq0n_l,
                  z0=np.asarray(z0, BF).reshape(1, 3, K0), w1_h=w1h, w1_l=w1l,
                  b1c=np.asarray(b1c, np.float32), w2_h=w2h, w2_l=w2l,
                  b2s=np.asarray(b2s, BF).reshape(1, 2, C2), vq1t_h=vq1t_h, vq1t_l=vq1t_l,
                  vq1n_h=np.asarray(vq1n_h, BF), vq1n_l=np.asarray(vq1n_l, BF),
                  z1=np.asarray(z1, BF).reshape(1, 3, NCODE))

    in_maps = []
    for c in range(N_CORES):
        xs = x0[c * IMGS:(c + 1) * IMGS].reshape(IMGS, 3, 128, NPX)
        xh, xxl = _split2(xs)
        m = dict(shared)
        m["xh"] = xh
        m["xl"] = xxl
        in_maps.append(m)
    return in_maps


def kernel(x0, vq0, vq1, w1, b1, w2, b2, cur_iter=None):
    from concourse.bass_utils import run_bass_kernel_spmd

    x0 = np.asarray(x0)
    nc = _program()
    in_maps = _host_inputs(x0, vq0, vq1, w1, b1, w2, b2)
    res = run_bass_kernel_spmd(nc, in_maps, list(range(N_CORES))).results

    def gather(name):
        return np.concatenate([res[c][name] for c in range(N_CORES)], axis=0)

    qx0 = gather("qx0o").reshape(-1, C1, H, W)
    a0 = gather("a0o").transpose(0, 2, 1, 3).reshape(-1, K0, H, W)
    x3 = gather("x3o").reshape(-1, C2, H, W)
    qx3 = gather("qx3o").transpose(0, 2, 1, 3).reshape(-1, C2, H, W)
    a3 = gather("a3o").reshape(-1, NCODE, H, W)
    return (np.asarray(x0, np.float32), x3, qx0, qx3, a0, a3)
